# revision 41
# baseline (speedup 1.0000x reference)
# Local (sliding-window, strictly-causal) multi-head attention for Trainium2.
#
# Problem: nn_LocalAttention  (B=2, S=4096, MD=AD=1024, NH=8, HD=128, window=256)
#   q = query @ Wq.T ; per-head scores q.k/sqrt(HD) masked to col in [row-256, row-1];
#   softmax; out = w @ v ; rows with no valid keys zeroed; out @ Wo.T.
#
# Sharding (8 cores): batch (2) x sequence chunks (4 x 1024 rows).  Each core runs
# the whole pipeline for its 1024 query rows using a 256-row K/V halo, so the 8
# output shards are disjoint and the gather is pure concatenation.  Weights are
# replicated.
#
# Device pipeline (v2):
#   - All wide matmuls (out free-dim >= 256) run as float32r: 1 cycle/row on the
#     PE array instead of fp32's 4.
#   - Scores are computed key-block-major: for each of the 10 halo key blocks,
#     one wide matmul produces scoresT[k, q] for the (up to) 3 query tiles that
#     attend to that block, on top of a mask bias preloaded into PSUM by a bf16
#     identity matmul.  The mask band is shift-invariant, so a single 384-column
#     interior bias tile is shared by key blocks 2..9 (edge blocks get their own
#     small tiles; the s0==0 halo-padding cases are baked into those per-core).
#   - exp() (no max subtraction: scores are O(1), masked entries -1e5 -> exp==0)
#     is written as bf16; the PV matmul and the 128x128 output transpose run in
#     bf16 (1 cycle/row).  V carries an interleaved ones column per head so the
#     softmax denominator falls out of the PV matmul; normalization is a
#     per-partition scalar multiply.
#   - The Wo projection accumulates all 8 heads into PSUM (f32r, 512-wide) and
#     streams out row-contiguous.

import math

import numpy as np

try:  # numpy bf16 via ml_dtypes (jax dependency, always present here)
    import ml_dtypes

    BF16_NP = np.dtype(ml_dtypes.bfloat16)
except ImportError:  # pragma: no cover
    BF16_NP = None

import concourse.bass as bass
import concourse.tile as tile
from concourse import bacc, mybir
from concourse.bass_utils import run_bass_kernel_spmd
from concourse.masks import make_identity

F32 = mybir.dt.float32
F32R = mybir.dt.float32r  # fast fp32 matmul mode: 1 cycle/row when out width >= 256
BF16 = mybir.dt.bfloat16

NH = 8       # heads
HD = 128     # head dim
B = 2        # batch
S = 4096     # sequence
MD = 1024    # model dim
AD = 1024    # attn dim
WIN = 256    # window
C = 1024     # query rows per core (chunk)
NQT = C // 128          # 8 query tiles per chunk
HALO = WIN + C          # 1280 key/value rows per core
NKB = HALO // 128       # 10 key blocks
VROW = NH * (HD + 1)    # 1032: v with a ones column interleaved per head
NCORES = 8
MASK_NEG = -1.0e5       # exp(-1e5 + O(1)) == 0 exactly in f32/bf16
EXP = mybir.ActivationFunctionType.Exp


# ----------------------------------------------------------------------------
# device program
# ----------------------------------------------------------------------------

def _emit(ctx, tc: tile.TileContext, qcT, wqT, woT, kT, vp, biasT, out):
    nc = tc.nc

    const_pool = ctx.enter_context(tc.tile_pool(name="const", bufs=1))
    ident = const_pool.tile([128, 128], BF16)
    make_identity(nc, ident)

    # pools that live for the whole kernel
    kT_pool = ctx.enter_context(tc.tile_pool(name="kT", bufs=1))
    bias_pool = ctx.enter_context(tc.tile_pool(name="bias", bufs=1))
    qT_pool = ctx.enter_context(tc.tile_pool(name="qT", bufs=1))

    kT_sb = kT_pool.tile([128, NH, HALO], BF16)
    # 0/1 mask, multiplied into exp(scores) by the Pool engine.  Columns:
    # [0:128) kb==0 (per-core) | [128:384) kb==1 | [384:1152) interior twice
    # (so interior key-block pairs can mask with one fused op; kb8 reads
    # [384:640), kb9 reads [384:512)).
    mask_sb = bias_pool.tile([128, 1152], BF16)
    qT_sb = qT_pool.tile([128, NH, C], BF16)

    # ---------------- phase 1: q projection -> qT[d, h, t] -------------------
    # mt-outer over head pairs so the PE consumes qc/wq chunks as they stream
    # in; two [128, 2, 1024] PSUM tiles (4 banks each) ping-pong.
    with tc.tile_pool(name="qc", bufs=1) as qc_pool, \
         tc.tile_pool(name="wq", bufs=1) as wq_pool, \
         tc.tile_pool(name="qp_psum", bufs=1, space="PSUM") as qp_psum:
        qc_sb = qc_pool.tile([128, 8, C], BF16)
        wq_sb = wq_pool.tile([128, 8, AD], BF16)
        # qc/wq chunks first (they gate everything).  The first head-quad only
        # needs wq columns 0:512, so those halves stream first; kT/bias/second
        # wq halves follow for the attention phase.
        for mt in range(8):
            nc.sync.dma_start(out=qc_sb[:, mt, :], in_=qcT[mt * 128:(mt + 1) * 128, :])
            nc.sync.dma_start(out=wq_sb[:, mt, 0:512],
                              in_=wqT[mt * 128:(mt + 1) * 128, 0:512])
        for mt in range(8):
            nc.sync.dma_start(out=wq_sb[:, mt, 512:1024],
                              in_=wqT[mt * 128:(mt + 1) * 128, 512:1024])
        for h in range(NH):
            nc.sync.dma_start(out=kT_sb[:, h, :], in_=kT[h])
        nc.sync.dma_start(out=mask_sb, in_=biasT)

        for gpair in range(2):          # (groups 0,1) then (groups 2,3)
            ps0 = qp_psum.tile([128, 2, C], F32)
            ps1 = qp_psum.tile([128, 2, C], F32)
            for mt in range(8):
                for g, ps in ((2 * gpair, ps0), (2 * gpair + 1, ps1)):
                    for hh in range(2):
                        h = 2 * g + hh
                        lhsT = wq_sb[:, mt, h * 128:(h + 1) * 128]
                        for nn in range(2):
                            nc.tensor.matmul(
                                ps[:, hh, nn * 512:(nn + 1) * 512],
                                lhsT=lhsT,
                                rhs=qc_sb[:, mt, nn * 512:(nn + 1) * 512],
                                start=(mt == 0),
                                stop=(mt == 7),
                            )
                    if mt == 7:   # copies start as soon as each group stops
                        for hh in range(2):   # split across ACT and DVE
                            dst = qT_sb[:, 2 * g + hh, :]
                            if hh == 0:
                                nc.scalar.copy(dst, ps[:, hh, :])
                            else:
                                nc.vector.tensor_copy(dst, ps[:, hh, :])

    # ---------------- phase 2: attention ------------------------------------
    with tc.tile_pool(name="vp", bufs=1) as vp_pool, \
         tc.tile_pool(name="wo", bufs=1) as wo_pool, \
         tc.tile_pool(name="outT", bufs=1) as outT_pool:

        vp_sb = vp_pool.tile([128, NKB, VROW], BF16)
        for blk in range(NKB):
            nc.sync.dma_start(out=vp_sb[:, blk, :], in_=vp[blk])
        wo_sb = wo_pool.tile([128, NH, MD], F32R)
        nc.sync.dma_start(out=wo_sb, in_=woT.rearrange("(h d) o -> d h o", d=128))
        outT_sb = outT_pool.tile([128, NH, NQT, 128], F32R)

        with tc.tile_pool(name="e", bufs=3) as e_pool, \
             tc.tile_pool(name="oh", bufs=4) as oh_pool, \
             tc.tile_pool(name="r", bufs=4) as r_pool, \
             tc.tile_pool(name="sc_psum", bufs=2, space="PSUM") as sc_psum, \
             tc.tile_pool(name="ov_psum", bufs=2, space="PSUM") as ov_psum, \
             tc.tile_pool(name="tr_psum", bufs=2, space="PSUM") as tr_psum:

            def emit_score_pair(h, e_sb, kb_a):
                # 512-wide slots: each matmul output must stay in one bank
                s_ps = sc_psum.tile([128, 2, 512], F32)
                ws = []
                for p in range(2):
                    kb = kb_a + p
                    qlo = max(0, kb - 2)
                    qhi = min(NQT - 1, kb)
                    w = (qhi - qlo + 1) * 128
                    ws.append(w)
                    nc.tensor.matmul(
                        s_ps[:, p, 0:w],
                        lhsT=kT_sb[:, h, kb * 128:(kb + 1) * 128],
                        rhs=qT_sb[:, h, qlo * 128:(qhi + 1) * 128],
                        start=True,
                        stop=True,
                    )
                if ws[0] == 384 and ws[1] == 384:   # interior pair: fused exp
                    nc.scalar.activation(
                        e_sb[:, kb_a:kb_a + 2, :], s_ps[:, :, 0:384], EXP)
                    # mask exp(scores) multiplicatively on the idle Pool
                    # engine (scores are O(1): no max subtraction needed)
                    esl = e_sb[:, kb_a:kb_a + 2, :].rearrange("p a b -> p (a b)")
                    nc.gpsimd.tensor_mul(esl, esl, mask_sb[:, 384:1152])
                else:
                    for p in range(2):
                        kb = kb_a + p
                        nc.scalar.activation(
                            e_sb[:, kb, 0:ws[p]], s_ps[:, p, 0:ws[p]], EXP)
                        if kb == 0:
                            msl = mask_sb[:, 0:128]
                        elif kb == 1:
                            msl = mask_sb[:, 128:384]
                        else:       # kb 8/9: prefix of the interior pattern
                            msl = mask_sb[:, 384:384 + ws[p]]
                        esl = e_sb[:, kb, 0:ws[p]]
                        nc.gpsimd.tensor_mul(esl, esl, msl)

            def emit_pv_pair(h, e_sb, qp):
                o_ps = ov_psum.tile([128, 2, HD + 1], F32)
                for j in range(2):
                    qt = 2 * qp + j
                    for sub in range(3):
                        kb = qt + sub
                        off = (qt - max(0, kb - 2)) * 128
                        nc.tensor.matmul(
                            o_ps[:, j, :],
                            lhsT=e_sb[:, kb, off:off + 128],
                            rhs=vp_sb[:, kb, h * (HD + 1):(h + 1) * (HD + 1)],
                            start=(sub == 0),
                            stop=(sub == 2),
                        )
                r_sb = r_pool.tile([128, 2], F32)
                nc.vector.reciprocal(
                    r_sb, o_ps[:, :, HD:HD + 1].rearrange("p a b -> p (a b)"))
                # both tiles normalized in one DVE op: 1/denominator broadcast
                # along the head dim
                oh_sb = oh_pool.tile([128, 2, 128], BF16)
                nc.vector.tensor_tensor(
                    oh_sb,
                    o_ps[:, :, 0:HD],
                    r_sb.unsqueeze(2).to_broadcast([128, 2, HD]),
                    mybir.AluOpType.mult,
                )
                t_ps = tr_psum.tile([128, 2, 128], BF16)
                for j in range(2):
                    nc.tensor.transpose(t_ps[:, j, :], oh_sb[:, j, :], ident)
                # keep ACT free for exp (the binding engine): copies on DVE
                nc.vector.tensor_copy(
                    outT_sb[:, h, 2 * qp:2 * qp + 2, :].rearrange("p a b -> p (a b)"),
                    t_ps.rearrange("p a b -> p (a b)"))

            # software pipeline: head h's scores/exp/mask interleave with
            # head h-1's PV, so the PE fills the gaps while ACT (exp, the
            # binding engine here) works through head h
            e_tiles = [None] * NH
            e_tiles[0] = e_pool.tile([128, NKB, 384], BF16, name="e_sb")
            for kb_a in range(0, NKB, 2):
                emit_score_pair(0, e_tiles[0], kb_a)
            for h in range(1, NH + 1):
                if h < NH:
                    e_tiles[h] = e_pool.tile([128, NKB, 384], BF16, name="e_sb")
                    for step in range(5):
                        emit_score_pair(h, e_tiles[h], 2 * step)
                        if step >= 1:
                            emit_pv_pair(h - 1, e_tiles[h - 1], step - 1)
                else:
                    for qp in range(NQT // 2):
                        emit_pv_pair(h - 1, e_tiles[h - 1], qp)

        # ---------------- phase 3: output projection -------------------------
        with tc.tile_pool(name="stage", bufs=2) as stage_pool, \
             tc.tile_pool(name="fi_psum", bufs=2, space="PSUM") as fi_psum:
            for qt in range(NQT):
                f_ps = fi_psum.tile([128, MD], F32)
                st = stage_pool.tile([128, MD], F32)
                # nn-outer: the first half's copy+DMA overlap the second
                # half's matmuls, shortening the end-of-kernel drain
                for nn in range(2):
                    for h in range(NH):
                        nc.tensor.matmul(
                            f_ps[:, nn * 512:(nn + 1) * 512],
                            lhsT=outT_sb[:, h, qt, :],
                            rhs=wo_sb[:, h, nn * 512:(nn + 1) * 512],
                            start=(h == 0),
                            stop=(h == NH - 1),
                        )
                    sl = slice(nn * 512, (nn + 1) * 512)
                    if nn == 0:
                        nc.scalar.copy(st[:, sl], f_ps[:, sl])
                    else:
                        nc.vector.tensor_copy(st[:, sl], f_ps[:, sl])
                    nc.sync.dma_start(
                        out=out[qt * 128:(qt + 1) * 128, sl], in_=st[:, sl])


_CACHED_NC = {}


def _build_program(iters: int = 1):
    if iters in _CACHED_NC:
        return _CACHED_NC[iters]
    nc = bacc.Bacc("TRN2", target_bir_lowering=False, debug=False)
    qcT = nc.dram_tensor("qcT", [MD, C], BF16, kind="ExternalInput").ap()
    wqT = nc.dram_tensor("wqT", [MD, AD], BF16, kind="ExternalInput").ap()
    woT = nc.dram_tensor("woT", [AD, MD], F32R, kind="ExternalInput").ap()
    kT = nc.dram_tensor("kT", [NH, HD, HALO], BF16, kind="ExternalInput").ap()
    vp = nc.dram_tensor("vp", [NKB, 128, VROW], BF16, kind="ExternalInput").ap()
    biasT = nc.dram_tensor("biasT", [128, 1152], BF16, kind="ExternalInput").ap()
    out = nc.dram_tensor("out", [C, MD], F32, kind="ExternalOutput").ap()
    from contextlib import ExitStack

    with tile.TileContext(nc) as tc:
        for _ in range(iters):
            with ExitStack() as ctx:
                _emit(ctx, tc, qcT, wqT, woT, kT, vp, biasT, out)
    nc.compile()
    _CACHED_NC[iters] = nc
    return nc


# ----------------------------------------------------------------------------
# host-side shard construction
# ----------------------------------------------------------------------------

def _build_mask(s0: int) -> np.ndarray:
    """0/1 mask, bf16, columns [kb0 | kb1 | interior x2]: [128, 1152].

    interior[k, j] (j = p*128 + c over the 3 query tiles kb-2..kb of any
    interior key block): valid iff 1 <= j - k <= WIN.  kb==0 stores query
    tile 0 only (j offset 256 of the interior pattern); kb==1 stores query
    tiles 0..1 (j offset 128).  For the s0==0 core, key blocks 0/1 sit in
    the zero-padded halo whose rows have a zeroed ones-column (so they
    can't pollute the softmax denominator) -- except element [0, 0] of
    kb0, which gives query row 0 one unmasked zero-valued key so its
    softmax output is exactly 0 (matching the reference's has_valid
    zeroing).
    """
    kk = np.arange(128)[:, None]
    jj = np.arange(384)[None, :]
    interior = ((jj - kk >= 1) & (jj - kk <= WIN)).astype(np.float32)

    m = np.empty((128, 1152), np.float32)
    if s0 == 0:
        m[:, 0:128] = 0.0
        m[0, 0] = 1.0
    else:
        m[:, 0:128] = interior[:, 256:384]
    m[:, 128:384] = interior[:, 128:384]
    m[:, 384:768] = interior
    m[:, 768:1152] = interior
    return m.astype(BF16_NP)


def _make_in_maps(query_seq, keys_seq, values_seq, Wq, Wo):
    q = np.ascontiguousarray(np.asarray(query_seq, dtype=np.float32))
    k = np.ascontiguousarray(np.asarray(keys_seq, dtype=np.float32))
    v = np.ascontiguousarray(np.asarray(values_seq, dtype=np.float32))
    wq = np.asarray(Wq, dtype=np.float32)
    wo = np.asarray(Wo, dtype=np.float32)

    scale = np.float32(math.sqrt(float(HD)))
    wqT = np.ascontiguousarray(wq.T / scale).astype(BF16_NP)
    woT = np.ascontiguousarray(wo.T)

    in_maps = []
    for core in range(NCORES):
        b, ch = divmod(core, S // C)
        s0 = ch * C

        qcT = np.ascontiguousarray(q[b, s0:s0 + C, :].T).astype(BF16_NP)  # [MD, C]

        khalo = np.zeros((HALO, AD), np.float32)
        vhalo = np.zeros((HALO, AD), np.float32)
        lo = s0 - WIN
        off = max(0, -lo)
        khalo[off:] = k[b, lo + off:s0 + C, :]
        vhalo[off:] = v[b, lo + off:s0 + C, :]

        kT = np.ascontiguousarray(
            khalo.reshape(HALO, NH, HD).transpose(1, 2, 0)).astype(BF16_NP)

        # ones column is zeroed on halo-padding rows so unmasked exp values
        # there can't pollute the softmax denominator (their v is 0 anyway);
        # row 0 of the s0==0 core keeps a single 1 for the has_valid trick.
        valid = np.zeros((HALO,), np.float32)
        valid[off:] = 1.0
        if s0 == 0:
            valid[0] = 1.0

        vp = np.zeros((NKB, 128, VROW), BF16_NP)
        vh = vhalo.reshape(NKB, 128, NH, HD)
        vones = valid.reshape(NKB, 128).astype(BF16_NP)
        for h in range(NH):
            vp[:, :, h * (HD + 1):h * (HD + 1) + HD] = vh[:, :, h, :].astype(BF16_NP)
            vp[:, :, h * (HD + 1) + HD] = vones

        in_maps.append({
            "qcT": qcT,
            "wqT": wqT,
            "woT": woT,
            "kT": kT,
            "vp": vp,
            "biasT": _build_mask(s0),
        })
    return in_maps


def _gather(results) -> np.ndarray:
    out = np.empty((B, S, MD), np.float32)
    for core in range(NCORES):
        b, ch = divmod(core, S // C)
        out[b, ch * C:(ch + 1) * C, :] = results[core]["out"]
    return out


def _run(in_maps, **kwargs):
    nc = _build_program()
    return run_bass_kernel_spmd(nc, in_maps, list(range(NCORES)), **kwargs)


def kernel(query_seq, keys_seq, values_seq, Wq, Wo, window=WIN, **_unused):
    assert int(window) == WIN, f"kernel hardcodes window={WIN}, got {window}"
    in_maps = _make_in_maps(query_seq, keys_seq, values_seq, Wq, Wo)
    res = _run(in_maps)
    return _gather(res.results)


def kernel_traced(query_seq, keys_seq, values_seq, Wq, Wo, window=WIN, **_unused):
    """Like kernel() but also returns BassKernelResults (profile/exec time)."""
    assert int(window) == WIN
    in_maps = _make_in_maps(query_seq, keys_seq, values_seq, Wq, Wo)
    res = _run(in_maps, trace=True)
    return _gather(res.results), res
